# revision 1
# baseline (speedup 1.0000x reference)
"""Trainium2 Bass kernel for nn_CustomLlamaAttention (partial-RoPE GQA attention
with low-rank KV, tensor-parallel over heads on 8 NeuronCores).

Self-contained: hardcodes shapes/sharding; builds one SPMD Bass program and runs
it on cores 0-7 via run_bass_kernel_spmd. Host pre-transposes/pre-casts inputs,
device computes everything transposed (feature-major), host re-assembles.

Sharding: core c owns q heads 4c..4c+3 (= KV head c), o_proj input-dim shard
c*256..(c+1)*256; hidden_states replicated; attn outputs AllGather'd on-device
before the o-projection; final [S,256] output shards concatenated on host.
"""

import sys

for _p in ("/opt/trn_rl_repo",):
    if _p not in sys.path:
        sys.path.append(_p)

import numpy as np
import ml_dtypes

import concourse.bass as bass
import concourse.tile as tile
from concourse import bacc
from concourse import mybir
from concourse.bass import ts
from concourse.bass_utils import run_bass_kernel_spmd

# ---- problem constants (hardcoded per spec) ----
HID = 2048
NH = 32
NKV = 8
HD = 64
LR = 32
TOPK = 16
THETA = 10000.0
B, S = 1, 2048
NCORES = 8
HPC = NH // NCORES          # 4 q heads per core
QSH = HPC * HD              # 256 q rows per core
KRSH = 2 * TOPK             # 32 roped dims per KV head
NOPESH = HD - KRSH          # 32 nope dims per KV head
LAT = LR * NKV              # 256 latent
WCAT = LAT + QSH + KRSH     # 544 = [down 256 | q 256 | kr 32]
P = 128
NSC = S // 512              # 4 s-chunks of 512
NKT = S // P                # 16 k-tiles of 128
BF = mybir.dt.bfloat16
F32 = mybir.dt.float32

# m-chunks of wcat columns: [down0, down1, q01, q23, kr]
MCHUNKS = [(0, 128), (128, 128), (256, 128), (384, 128), (512, 32)]

USE_PACK = True  # row-packed scores matmuls (2 heads per PE pass)


def _build_program():
    nc = bacc.Bacc(
        "TRN2",
        target_bir_lowering=False,
        debug=False,
        num_devices=NCORES,
    )

    # DRAM I/O (per-core data supplied via in_maps)
    hsT_d = nc.dram_tensor("hsT", [HID, S], BF, kind="ExternalInput").ap()
    wcatT_d = nc.dram_tensor("wcatT", [HID, WCAT], BF, kind="ExternalInput").ap()
    upkT_d = nc.dram_tensor("upkT", [LAT, NOPESH], BF, kind="ExternalInput").ap()
    upvT_d = nc.dram_tensor("upvT", [LAT, HD], BF, kind="ExternalInput").ap()
    owT_d = nc.dram_tensor("owT", [HID, QSH], BF, kind="ExternalInput").ap()
    cosq_d = nc.dram_tensor("cosq", [P, S], BF, kind="ExternalInput").ap()
    sinq_d = nc.dram_tensor("sinq", [P, S], BF, kind="ExternalInput").ap()
    cosk_d = nc.dram_tensor("cosk", [KRSH, S], BF, kind="ExternalInput").ap()
    sink_d = nc.dram_tensor("sink", [KRSH, S], BF, kind="ExternalInput").ap()
    mask_d = nc.dram_tensor("maskdiag", [P, 4, 512], BF, kind="ExternalInput").ap()

    outT_d = nc.dram_tensor("outT", [QSH, S], F32, kind="ExternalOutput").ap()

    # internal DRAM for the attention-output AllGather
    og_in = nc.dram_tensor("og_in", [QSH, S], BF).ap()
    og_out = nc.dram_tensor("og_out", [NH * HD, S], BF, addr_space="Shared").ap()

    Exp = mybir.ActivationFunctionType.Exp

    with tile.TileContext(nc) as tc:
        with (
            tc.tile_pool(name="sing", bufs=1) as sing,
            tc.tile_pool(name="tmp", bufs=3) as tmp,
            tc.tile_pool(name="apool", bufs=4) as apool,
            tc.tile_pool(name="psum_mm", bufs=2, space="PSUM") as psum_mm,
            tc.tile_pool(name="psum_s", bufs=4, space="PSUM") as psum_s,
            tc.tile_pool(name="psum_av", bufs=2, space="PSUM") as psum_av,
        ):
            # ---- persistent SBUF tiles ----
            big = sing.tile([P, NKT, S], BF, tag="big")  # hsT now, OT later
            wcat_sb = sing.tile([P, NKT, WCAT], BF, tag="wcat")
            ow_sb = sing.tile([P, NKT, QSH], BF, tag="ow")
            upk_sb = sing.tile([P, 2, NOPESH], BF, tag="upk")
            upv_sb = sing.tile([P, 2, HD], BF, tag="upv")
            cosq_sb = sing.tile([P, S], BF, tag="cosq")
            sinq_sb = sing.tile([P, S], BF, tag="sinq")
            cosk_sb = sing.tile([KRSH, S], BF, tag="cosk")
            sink_sb = sing.tile([KRSH, S], BF, tag="sink")
            mask_sb = sing.tile([P, 4, 512], BF, tag="mask")
            yT = sing.tile([P, 5, S], BF, tag="yT")
            KT = sing.tile([P, NKT, P], BF, tag="KT")   # rows: 2 dup bands of 64 d
            V = sing.tile([P, NKT, HD + 1], BF, tag="V")  # col HD = ones
            qr0 = sing.tile([P, S], BF, tag="qr0")      # heads 0,1 (rope'd)
            qr1 = sing.tile([P, S], BF, tag="qr1")      # heads 2,3
            krot = sing.tile([KRSH, S], BF, tag="krot")

            # ---- loads ----
            nc.sync.dma_start(out=big, in_=hsT_d.rearrange("(ko p) s -> p ko s", p=P))
            nc.sync.dma_start(
                out=wcat_sb, in_=wcatT_d.rearrange("(ko p) m -> p ko m", p=P)
            )
            nc.sync.dma_start(out=ow_sb, in_=owT_d.rearrange("(ko p) m -> p ko m", p=P))
            nc.sync.dma_start(out=upk_sb, in_=upkT_d.rearrange("(ko p) m -> p ko m", p=P))
            nc.sync.dma_start(out=upv_sb, in_=upvT_d.rearrange("(ko p) m -> p ko m", p=P))
            nc.sync.dma_start(out=cosq_sb, in_=cosq_d)
            nc.sync.dma_start(out=sinq_sb, in_=sinq_d)
            nc.sync.dma_start(out=cosk_sb, in_=cosk_d)
            nc.sync.dma_start(out=sink_sb, in_=sink_d)
            nc.sync.dma_start(out=mask_sb, in_=mask_d)

            nc.vector.memset(V[:, :, HD : HD + 1], 1.0)

            # ---- phase 1: fused projection  y^T = wcat @ hs^T ----
            for mi, (m0, msz) in enumerate(MCHUNKS):
                for scj in range(NSC):
                    ps = psum_mm.tile([P, 512], F32, tag="mm")
                    for kt in range(NKT):
                        nc.tensor.matmul(
                            ps[:msz],
                            lhsT=wcat_sb[:, kt, m0 : m0 + msz],
                            rhs=big[:, kt, ts(scj, 512)],
                            start=(kt == 0),
                            stop=(kt == NKT - 1),
                        )
                    nc.vector.tensor_copy(
                        out=yT[:msz, mi, ts(scj, 512)], in_=ps[:msz]
                    )

            # ---- phase 2a: up-projections ----
            # k_c^T (nope key rows) -> aligned staging tile, then DMA into KT
            kcT = sing.tile([NOPESH, NKT, P], BF, tag="kcT")
            for scj in range(NSC):
                ps = psum_mm.tile([P, 512], F32, tag="mm")
                for lt in range(2):
                    nc.tensor.matmul(
                        ps[:NOPESH],
                        lhsT=upk_sb[:, lt, :],
                        rhs=yT[:, lt, ts(scj, 512)],
                        start=(lt == 0),
                        stop=(lt == 1),
                    )
                nc.vector.tensor_copy(
                    out=kcT[:, ts(scj, 4), :],
                    in_=ps[:NOPESH].rearrange("p (ko ki) -> p ko ki", ki=P),
                )
            for b in (0, 64):
                # nope dims 16:32 and 48:64 within each 64-row d band
                nc.sync.dma_start(out=KT[b + 16 : b + 32, :, :], in_=kcT[0:16])
                nc.sync.dma_start(out=KT[b + 48 : b + 64, :, :], in_=kcT[16:32])

            # V (s on partitions): V[s,:] = c_kv[s,:] @ upv^T
            for m in range(NKT):
                ps = psum_mm.tile([P, 512], F32, tag="mm")
                for lt in range(2):
                    nc.tensor.matmul(
                        ps[:, 0:HD],
                        lhsT=yT[:, lt, ts(m, P)],
                        rhs=upv_sb[:, lt, :],
                        start=(lt == 0),
                        stop=(lt == 1),
                    )
                nc.vector.tensor_copy(out=V[:, m, 0:HD], in_=ps[:, 0:HD])

            # ---- phase 2b: RoPE ----
            # q (both 2-head groups), scale 1/sqrt(HD) folded into tables
            for g, dst in ((2, qr0), (3, qr1)):
                qt = yT[:, g, :]
                qsh_t = tmp.tile([P, S], BF, tag="qsh")
                for b in (0, 64):
                    nc.vector.tensor_copy(out=qsh_t[b : b + 32], in_=qt[b + 32 : b + 64])
                    nc.vector.tensor_copy(out=qsh_t[b + 32 : b + 64], in_=qt[b : b + 32])
                nc.vector.tensor_mul(out=dst, in0=qt, in1=cosq_sb)
                nc.vector.tensor_mul(out=qsh_t, in0=qsh_t, in1=sinq_sb)
                nc.vector.tensor_add(out=dst, in0=dst, in1=qsh_t)

            # k_r rows: krT = yT[0:32, 4, :], pairs are (r, r+16)
            krT = yT[0:KRSH, 4, :]
            ksh_t = tmp.tile([KRSH, S], BF, tag="ksh")
            nc.sync.dma_start(out=ksh_t[0:16], in_=krT[16:32])
            nc.sync.dma_start(out=ksh_t[16:32], in_=krT[0:16])
            nc.vector.tensor_mul(out=krot, in0=krT, in1=cosk_sb)
            nc.vector.tensor_mul(out=ksh_t, in0=ksh_t, in1=sink_sb)
            nc.vector.tensor_add(out=krot, in0=krot, in1=ksh_t)

            # scatter rope'd k rows into KT bands (d 0:16 and 32:48)
            for b in (0, 64):
                nc.sync.dma_start(
                    out=KT[b + 0 : b + 16, :, :],
                    in_=krot[0:16].rearrange("p (ko ki) -> p ko ki", ki=P),
                )
                nc.sync.dma_start(
                    out=KT[b + 32 : b + 48, :, :],
                    in_=krot[16:32].rearrange("p (ko ki) -> p ko ki", ki=P),
                )

            # ---- phase 3: causal attention, 4 local heads ----
            for j in range(NSC):  # q-chunk of 512
                nkt = 4 * j + 4   # causal k-tiles
                for p_i, qr in enumerate((qr0, qr1)):
                    av = [
                        psum_av.tile([P, 512], F32, tag="av", name=f"av0_{j}_{p_i}"),
                        psum_av.tile([P, 512], F32, tag="av", name=f"av1_{j}_{p_i}"),
                    ]
                    for kt in range(nkt):
                        ss = [
                            psum_s.tile([P, 512], F32, tag="s", name=f"s0_{j}_{p_i}_{kt}"),
                            psum_s.tile([P, 512], F32, tag="s", name=f"s1_{j}_{p_i}_{kt}"),
                        ]
                        for hb, b0 in ((0, 0), (1, 64)):
                            nc.tensor.matmul(
                                ss[hb],
                                lhsT=KT[b0 : b0 + 64, kt, :],
                                rhs=qr[b0 : b0 + 64, ts(j, 512)],
                                start=True,
                                stop=True,
                            )
                        for hb in (0, 1):
                            a = apool.tile([P, 512], BF, tag="a")
                            nc.scalar.activation(a, ss[hb], Exp)
                            if kt >= 4 * j:
                                nc.vector.tensor_mul(
                                    out=a, in0=a, in1=mask_sb[:, kt - 4 * j, :]
                                )
                            nc.tensor.matmul(
                                av[hb][0 : HD + 1],
                                lhsT=V[:, kt, :],
                                rhs=a,
                                start=(kt == 0),
                                stop=(kt == nkt - 1),
                            )
                    for hb in (0, 1):
                        h = 2 * p_i + hb
                        rc = tmp.tile([1, 512], F32, tag="rc")
                        nc.vector.reciprocal(rc, av[hb][HD : HD + 1, :])
                        bc = tmp.tile([HD, 512], F32, tag="bc")
                        nc.gpsimd.partition_broadcast(bc, rc, channels=HD)
                        on = tmp.tile([HD, 512], BF, tag="on")
                        nc.vector.tensor_mul(out=on, in0=av[hb][0:HD, :], in1=bc)
                        nc.sync.dma_start(
                            out=og_in[h * HD : (h + 1) * HD, ts(j, 512)], in_=on
                        )

            # ---- phase 4: AllGather attention outputs across cores ----
            nc.gpsimd.collective_compute(
                "AllGather",
                mybir.AluOpType.bypass,
                replica_groups=[list(range(NCORES))],
                ins=[og_in],
                outs=[og_out],
            )

            # ---- phase 5: o-projection shard ----
            OT = sing.tile([P, NKT, S], BF, tag="big")  # reuse hsT slot
            nc.sync.dma_start(out=OT, in_=og_out.rearrange("(ko p) s -> p ko s", p=P))
            for mc in range(QSH // P):
                for scj in range(NSC):
                    ps = psum_mm.tile([P, 512], F32, tag="mm")
                    for kt in range(NKT):
                        nc.tensor.matmul(
                            ps,
                            lhsT=ow_sb[:, kt, ts(mc, P)],
                            rhs=OT[:, kt, ts(scj, 512)],
                            start=(kt == 0),
                            stop=(kt == NKT - 1),
                        )
                    ot = tmp.tile([P, 512], F32, tag="out")
                    nc.vector.tensor_copy(out=ot, in_=ps)
                    nc.sync.dma_start(
                        out=outT_d[ts(mc, P), ts(scj, 512)], in_=ot
                    )

    nc.compile()
    return nc


_NC_CACHE = None


def _get_program():
    global _NC_CACHE
    if _NC_CACHE is None:
        _NC_CACHE = _build_program()
    return _NC_CACHE


def _bf16(x):
    return np.asarray(x, dtype=np.float32).astype(ml_dtypes.bfloat16)


def _host_inputs(hidden_states, q_w, kr_w, down_w, upk_w, upv_w, o_w):
    hs = np.asarray(hidden_states, dtype=np.float32)[0]  # [S, HID]
    q_w = np.asarray(q_w, np.float32)
    kr_w = np.asarray(kr_w, np.float32)
    down_w = np.asarray(down_w, np.float32)
    upk_w = np.asarray(upk_w, np.float32)
    upv_w = np.asarray(upv_w, np.float32)
    o_w = np.asarray(o_w, np.float32)

    hsT = _bf16(hs.T)  # [HID, S]

    # RoPE tables (fp32 host math, bf16 on device)
    pos = np.arange(S, dtype=np.float32)
    inv = 1.0 / (THETA ** (np.arange(0, HD, 2, dtype=np.float32) / HD))
    fr = pos[:, None] * inv[None, :]           # [S, 32]
    emb = np.concatenate([fr, fr], -1)         # [S, 64]
    cosT = np.cos(emb).T                       # [64, S]
    sinT = np.sin(emb).T
    sc = 1.0 / np.sqrt(np.float32(HD))

    cosq = np.tile(cosT, (2, 1)) * sc          # [128, S]
    sgn = np.where(np.arange(HD) < 32, -1.0, 1.0).astype(np.float32)[:, None]
    sinq = np.tile(sinT * sgn, (2, 1)) * sc    # [128, S]

    rope_d = np.concatenate([np.arange(0, 16), np.arange(32, 48)])
    cosk = cosT[rope_d]                        # [32, S]
    sgnk = np.where(np.arange(KRSH) < 16, -1.0, 1.0).astype(np.float32)[:, None]
    sink = sinT[rope_d] * sgnk

    # diagonal causal masks for the 4 k-tile offsets within a 512 q-chunk
    kk = np.arange(P)[:, None]
    qq = np.arange(512)[None, :]
    mask = np.stack(
        [(P * i + kk <= qq).astype(np.float32) for i in range(4)], axis=1
    )  # [128, 4, 512]

    shared = {
        "hsT": hsT,
        "cosq": _bf16(cosq),
        "sinq": _bf16(sinq),
        "cosk": _bf16(cosk),
        "sink": _bf16(sink),
        "maskdiag": _bf16(mask),
    }
    in_maps = []
    for c in range(NCORES):
        q_rows = q_w[c * QSH : (c + 1) * QSH]          # [256, HID]
        kr_rows = kr_w[c * KRSH : (c + 1) * KRSH]      # [32, HID]
        wcat = np.concatenate([down_w, q_rows, kr_rows], axis=0)  # [544, HID]
        m = dict(shared)
        m["wcatT"] = _bf16(wcat.T)                     # [HID, 544]
        m["upkT"] = _bf16(upk_w[c * NOPESH : (c + 1) * NOPESH].T)  # [256, 32]
        m["upvT"] = _bf16(upv_w[c * HD : (c + 1) * HD].T)          # [256, 64]
        m["owT"] = _bf16(o_w[c * QSH : (c + 1) * QSH].T)           # [HID, 256]
        in_maps.append(m)
    return in_maps


def kernel(**inputs) -> np.ndarray:
    nc = _get_program()
    in_maps = _host_inputs(**inputs)
    res = run_bass_kernel_spmd(nc, in_maps, core_ids=list(range(NCORES)))
    outT = np.concatenate(
        [np.asarray(res.results[c]["outT"]) for c in range(NCORES)], axis=0
    )  # [2048, S]
    return np.ascontiguousarray(outT.T)[None].astype(np.float32)


if __name__ == "__main__":
    rng = np.random.default_rng(0)
    ins = {
        "hidden_states": rng.standard_normal((B, S, HID), dtype=np.float32),
        "q_w": rng.standard_normal((NH * HD, HID), dtype=np.float32) * 0.02,
        "kr_w": rng.standard_normal((2 * TOPK * NKV, HID), dtype=np.float32) * 0.02,
        "down_w": rng.standard_normal((LAT, HID), dtype=np.float32) * 0.02,
        "upk_w": rng.standard_normal((NOPESH * NKV, LAT), dtype=np.float32) * 0.02,
        "upv_w": rng.standard_normal((NKV * HD, LAT), dtype=np.float32) * 0.02,
        "o_w": rng.standard_normal((HID, NH * HD), dtype=np.float32) * 0.02,
    }
    out = kernel(**ins)
    print(out.shape, out.dtype, float(np.abs(out).max()))

